# revision 17
# baseline (speedup 1.0000x reference)
"""Trainium2 Bass kernel for nn_CMIA_2843268350555 (dual-branch spatial/freq attention).

Strategy: data-parallel over batch (16 samples / 8 cores = 2 per core).
Weights resident in SBUF; big weights (wqk, w_spa, w_frq) in bf16 so they
fit and DMA fast; activations mostly f32r.

Per-sample math (C=256 channels, HW=1024):
  vT_b    = (x_b.T @ w_bv.T)            [hw, c]   (b in {spa, frq})
  x       = w_cdc @ [x_spa; x_frq]      [c, hw]   (+b_cdc: no-op through LN)
  xn      = layernorm_rows(x)           [c, hw]   (affine folded into wqkTg)
  xnT     = transpose(xn)               [hw, c]
  q       = xn @ wqk_q                  [c, hw]   (lhsT=xnT chunks)
  kT      = wqk_k.T @ xn.T              [hw, c]   (lhsT=wqk chunks, rhs=xnT)
  kw_b    = (kT.T @ (scale*w_b.T))      [c, hw]
  logits  = q.T @ kw_b                  [hw(n), hw(j)]
  att_b   = softmax_j(logits + b_b)     (1/rowsum folded into vT)
  out_b   = x_b + (vT_b.T @ att_b)      [c, hw]

Schedule (per 2-sample iteration), interleaved to keep PE dense:
  A0 B0 LN0 A1 B1 LN1 C0 Dq0 Dk0 | E0s F0s E0f G0s F0f | C1 Dq1 Dk1 G0f |
  E1s F1s E1f G1s F1f G1f
PSUM: pool psS 2x[128,512] (A/B/C) + psB 3x[128,1024] (Dq/Dk/E/F/G) = 8 banks.
"""
import numpy as np
import ml_dtypes

import concourse.bacc as bacc
import concourse.mybir as mybir
import concourse.tile as tile
from concourse import bass_utils
from concourse.bass import ts, ds
from concourse.masks import make_identity

f32 = mybir.dt.float32
f32r = mybir.dt.float32r
bf16 = mybir.dt.bfloat16
u32 = mybir.dt.uint32

B, C, H, W = 16, 256, 32, 32
HW = H * W           # 1024
J2 = 2 * HW          # 2048
NCORES = 8
BPC = B // NCORES    # samples per core
CC = C // 128        # 2 channel chunks
NCH = HW // 128      # 8 hw chunks
EPS = 1e-5

# rstd via DVE pow + one Newton step (no Act Sqrt table load). Fallback
# LN_POW=False uses scalar-engine Sqrt (costs act-table swaps vs Exp).
LN_POW = True


def _round_f32r(x: np.ndarray) -> np.ndarray:
    """RNE-round fp32 to fp32r (11 mantissa bits; low 12 bits zero)."""
    x = np.ascontiguousarray(x, dtype=np.float32)
    u = x.view(np.uint32)
    lsb = (u >> np.uint32(12)) & np.uint32(1)
    r = u + np.uint32(0x7FF) + lsb
    return (r & ~np.uint32(0xFFF)).view(np.float32)


def _bf16(x: np.ndarray) -> np.ndarray:
    return np.ascontiguousarray(x, np.float32).astype(ml_dtypes.bfloat16)


_CACHE: dict = {}


def _build(flags, reps=1):
    has_qkb, has_bspa, has_bfrq, has_bsv, has_bfv = flags
    any_mm_bias = has_qkb or has_bspa or has_bfrq or has_bsv or has_bfv

    nc = bacc.Bacc("TRN2", target_bir_lowering=False, debug=False,
                   enable_asserts=True, num_devices=NCORES)
    xs_d = nc.dram_tensor("xs", [BPC, C, HW], f32r, kind="ExternalInput").ap()
    xf_d = nc.dram_tensor("xf", [BPC, C, HW], f32r, kind="ExternalInput").ap()
    wcdc_d = nc.dram_tensor("wcdcT", [2 * C, C], f32r, kind="ExternalInput").ap()
    wsv_d = nc.dram_tensor("wsvT", [C, C], f32r, kind="ExternalInput").ap()
    wfv_d = nc.dram_tensor("wfvT", [C, C], f32r, kind="ExternalInput").ap()
    wqk_d = nc.dram_tensor("wqkTg", [HW, J2], bf16, kind="ExternalInput").ap()
    wspa_d = nc.dram_tensor("wspaT", [HW, HW], bf16, kind="ExternalInput").ap()
    wfrq_d = nc.dram_tensor("wfrqT", [HW, HW], bf16, kind="ExternalInput").ap()
    qkb_d = qkbk_d = bspa_d = bfrq_d = bsv_d = bfv_d = None
    if has_qkb:
        qkb_d = nc.dram_tensor("qkb", [1, HW], f32r, kind="ExternalInput").ap()
        qkbk_d = nc.dram_tensor("qkbk", [128, NCH], f32, kind="ExternalInput").ap()
    if has_bspa:
        bspa_d = nc.dram_tensor("bspa", [1, HW], f32r, kind="ExternalInput").ap()
    if has_bfrq:
        bfrq_d = nc.dram_tensor("bfrq", [1, HW], f32r, kind="ExternalInput").ap()
    if has_bsv:
        bsv_d = nc.dram_tensor("bsv", [1, C], f32r, kind="ExternalInput").ap()
    if has_bfv:
        bfv_d = nc.dram_tensor("bfv", [1, C], f32r, kind="ExternalInput").ap()
    os_d = nc.dram_tensor("os", [BPC, C, HW], f32, kind="ExternalOutput").ap()
    of_d = nc.dram_tensor("of", [BPC, C, HW], f32, kind="ExternalOutput").ap()

    Sqrt = mybir.ActivationFunctionType.Sqrt
    Exp = mybir.ActivationFunctionType.Exp
    SUB = mybir.AluOpType.subtract
    MUL = mybir.AluOpType.mult
    ADD = mybir.AluOpType.add
    POW = mybir.AluOpType.pow

    with tile.TileContext(nc) as tc:
        with tc.tile_pool(name="constp", bufs=1) as constp, \
             tc.tile_pool(name="wqkp", bufs=1) as wqkp, \
             tc.tile_pool(name="wsp", bufs=1) as wsp, \
             tc.tile_pool(name="xin", bufs=2) as xin, \
             tc.tile_pool(name="dat", bufs=2) as dat, \
             tc.tile_pool(name="attp", bufs=10) as attp, \
             tc.tile_pool(name="resp", bufs=2) as resp, \
             tc.tile_pool(name="small", bufs=4) as small, \
             tc.tile_pool(name="psS", bufs=2, space="PSUM") as psS, \
             tc.tile_pool(name="psB", bufs=3, space="PSUM") as psB:

            # ---- resident weights ----
            # Queue plan: Pool(SWDGE) = small weights, A/B-critical first;
            # ACT = wqk chunks (needed from stage D); SP = inputs then ws.
            wsv_t = constp.tile([128, CC, C], f32r, name="wsv_t")
            nc.scalar.dma_start(out=wsv_t,
                                in_=wsv_d.rearrange("(kc p) c -> p kc c", p=128))
            wfv_t = constp.tile([128, CC, C], f32r, name="wfv_t")
            nc.scalar.dma_start(out=wfv_t,
                                in_=wfv_d.rearrange("(kc p) c -> p kc c", p=128))
            wcdc_t = constp.tile([128, 4, C], f32r, name="wcdc_t")
            nc.scalar.dma_start(out=wcdc_t,
                                in_=wcdc_d.rearrange("(kc p) c -> p kc c", p=128))
            ident = constp.tile([128, 128], bf16, name="ident")
            make_identity(nc, ident)
            ones_t = None
            if any_mm_bias:
                ones_f = constp.tile([1, 128], f32, name="ones_f")
                nc.vector.memset(ones_f, 1.0)
                ones_t = constp.tile([1, 128], f32r, name="ones_t")
                nc.scalar.copy(out=ones_t, in_=ones_f)

            def _bias_tile(dram, n, nm):
                t = constp.tile([1, n], f32r, name=nm)
                nc.gpsimd.dma_start(out=t, in_=dram)
                return t

            qkb_t = _bias_tile(qkb_d, HW, "qkb_t") if has_qkb else None
            bspa_t = _bias_tile(bspa_d, HW, "bspa_t") if has_bspa else None
            bfrq_t = _bias_tile(bfrq_d, HW, "bfrq_t") if has_bfrq else None
            bsv_t = _bias_tile(bsv_d, C, "bsv_t") if has_bsv else None
            bfv_t = _bias_tile(bfv_d, C, "bfv_t") if has_bfv else None
            qkbk_t = None
            if has_qkb:
                qkbk_t = constp.tile([128, NCH], f32, name="qkbk_t")
                nc.gpsimd.dma_start(out=qkbk_t, in_=qkbk_d)

            def _copy(eng, out, in_):
                if eng is nc.scalar:
                    eng.copy(out=out, in_=in_)
                else:
                    eng.tensor_copy(out=out, in_=in_)

            # Big weights all on the Pool/SWDGE queue: its SEQ has no early
            # compute, so the long dispatch burst can't clog Act/DVE/SP.
            wqk_t = wqkp.tile([128, NCH, J2], bf16, name="wqk_t")
            for kc in range(NCH):
                nc.gpsimd.dma_start(out=wqk_t[:, kc, :],
                                    in_=wqk_d[ds(kc * 128, 128), :])

            # ws resident, chunked so stage E's region deps land early
            wspa_t = wsp.tile([128, NCH, HW], bf16, name="wspa_t")
            wfrq_t = wsp.tile([128, NCH, HW], bf16, name="wfrq_t")
            for t, d in ((wspa_t, wspa_d), (wfrq_t, wfrq_d)):
                for mc in range(NCH):
                    nc.gpsimd.dma_start(out=t[:, mc, :],
                                        in_=d[ds(mc * 128, 128), :])

            def _samples_body():
                xsl, xfl, vtsl, vtfl, xsbl, xnTl, ql, kTl = \
                    [], [], [], [], [], [], [], []
                for b in range(BPC):
                    # sample 0 inputs on SP, sample 1 on ACT: two queues race
                    # ahead instead of one serial stream behind the weights
                    ieng = nc.sync if b == 0 else nc.scalar
                    xs_t = xin.tile([128, CC, HW], f32r, tag="xs", name=f"xs{b}")
                    ieng.dma_start(
                        out=xs_t,
                        in_=xs_d[b].rearrange("(cc p) n -> p cc n", p=128))
                    xf_t = xin.tile([128, CC, HW], f32r, tag="xf", name=f"xf{b}")
                    ieng.dma_start(
                        out=xf_t,
                        in_=xf_d[b].rearrange("(cc p) n -> p cc n", p=128))
                    xsl.append(xs_t)
                    xfl.append(xf_t)
                    vtsl.append(dat.tile([128, NCH, C], bf16, tag="vts",
                                         name=f"vts{b}"))
                    vtfl.append(dat.tile([128, NCH, C], bf16, tag="vtf",
                                         name=f"vtf{b}"))
                    xsbl.append(dat.tile([128, CC, HW], bf16, tag="xsb",
                                         name=f"xsb{b}"))
                    xnTl.append(dat.tile([128, NCH, C], bf16, tag="xnT",
                                         name=f"xnT{b}"))
                    ql.append(dat.tile([128, CC, HW], bf16, tag="q",
                                       name=f"q{b}"))
                    kTl.append(dat.tile([128, NCH, C], bf16, tag="kT",
                                        name=f"kT{b}"))
                kwl = [dat.tile([128, CC, HW], bf16, tag=f"kw{br}", bufs=1,
                                name=f"kw{br}") for br in range(2)]

                def stageA(b):
                    # value projections vT_b = x_b.T @ w_bv.T  [hw, c]
                    # two mc chunks share one PSUM tile -> one grouped copy
                    for i, (src, wv, dst, bt) in enumerate(
                            ((xsl[b], wsv_t, vtsl[b], bsv_t),
                             (xfl[b], wfv_t, vtfl[b], bfv_t))):
                        for mg in range(NCH // 2):
                            ps = psS.tile([128, 512], f32, tag="ps", name="psa")
                            for half in range(2):
                                mc = mg * 2 + half
                                for kc in range(CC):
                                    nc.tensor.matmul(
                                        ps[:, ds(half * C, C)],
                                        src[:, kc, ts(mc, 128)], wv[:, kc, :],
                                        start=(kc == 0),
                                        stop=(kc == CC - 1 and bt is None))
                                if bt is not None:
                                    nc.tensor.matmul(ps[:, ds(half * C, C)],
                                                     ones_t, bt,
                                                     start=False, stop=True)
                            eng = nc.vector if mg % 2 == 0 else nc.scalar
                            _copy(eng, dst[:, ds(mg * 2, 2), :],
                                  ps.rearrange("p (a f) -> p a f", a=2))

                def stageB(b):
                    # x = w_cdc @ [xs; xf]  [c, hw] -> bf16 x_sb
                    for cc in range(CC):
                        pb = psB.tile([128, HW], f32, tag="pb", name="psb")
                        for nn in range(2):
                            for kc in range(4):
                                src = xsl[b] if kc < 2 else xfl[b]
                                nc.tensor.matmul(
                                    pb[:, ds(nn * 512, 512)],
                                    wcdc_t[:, kc, ts(cc, 128)],
                                    src[:, kc % 2, ds(nn * 512, 512)],
                                    start=(kc == 0), stop=(kc == 3))
                        eng = nc.scalar if cc == 0 else nc.vector
                        _copy(eng, xsbl[b][:, cc, :], pb)

                def stageLN(b):
                    # rstd = 1/sqrt(var+eps) on DVE only: Newton iteration
                    # y' = y*(1.5 - 0.5*v*y^2), seeded y0 = min(1/v, 1) so
                    # v*y0^2 <= 1 < 3 (convergent for any v > 0). Avoids the
                    # Act-engine Sqrt table load that thrashes against Exp.
                    x_sb = xsbl[b]
                    mvs = []
                    for cc in range(CC):
                        xr = x_sb[:, cc, :].rearrange("p (s f) -> p s f", s=2)
                        stats = small.tile([128, 2, 6], f32, tag="st",
                                           name="stats")
                        for s in range(2):
                            nc.vector.bn_stats(out=stats[:, s, :],
                                               in_=xr[:, s, :])
                        mv = small.tile([128, 2], f32, tag=f"mv{cc}",
                                        name="mv")
                        nc.vector.bn_aggr(out=mv, in_=stats)
                        mvs.append(mv)
                    veps = small.tile([128, 2], f32, tag="veps", name="veps")
                    for cc in range(CC):
                        nc.vector.tensor_scalar(
                            out=veps[:, cc:cc + 1], in0=mvs[cc][:, 1:2],
                            scalar1=EPS, scalar2=None, op0=ADD)
                    y = small.tile([128, 2], f32, tag="rstd", name="rstd")
                    nc.vector.reciprocal(out=y, in_=veps)
                    nc.vector.tensor_scalar_min(out=y, in0=y, scalar1=1.0)
                    t = small.tile([128, 2], f32, tag="nt", name="nt")
                    for _ in range(5):
                        nc.vector.tensor_tensor(out=t, in0=y, in1=y, op=MUL)
                        nc.vector.tensor_tensor(out=t, in0=t, in1=veps,
                                                op=MUL)
                        nc.vector.tensor_scalar(
                            out=t, in0=t, scalar1=-0.5, scalar2=1.5,
                            op0=MUL, op1=ADD)
                        nc.vector.tensor_tensor(out=y, in0=y, in1=t, op=MUL)
                    for cc in range(CC):
                        nc.vector.tensor_scalar(
                            out=x_sb[:, cc, :], in0=x_sb[:, cc, :],
                            scalar1=mvs[cc][:, 0:1], scalar2=y[:, cc:cc + 1],
                            op0=SUB, op1=MUL)

                def stageC(b):
                    # xnT = xn.T: 4 transposes per PSUM tile, 1 grouped copy
                    for cc in range(CC):
                        for dg in range(2):
                            pt = psS.tile([128, 512], f32, tag="ps", name="pt")
                            ptb = pt.bitcast(bf16)
                            for i in range(4):
                                dc = dg * 4 + i
                                nc.tensor.transpose(
                                    ptb[:, ds(i * 128, 128)],
                                    xsbl[b][:, cc, ds(dc * 128, 128)], ident)
                            eng = nc.scalar if dg == 0 else nc.vector
                            _copy(eng,
                                  xnTl[b][:, ds(dg * 4, 4), ts(cc, 128)],
                                  ptb[:, 0:512].rearrange(
                                      "p (a f) -> p a f", a=4))

                def stageDq(b):
                    # q = xn @ wqk_q  [c, hw]
                    for cc in range(CC):
                        pb = psB.tile([128, HW], f32, tag="pb", name="psq")
                        for dc in range(NCH):
                            for nn in range(2):
                                nc.tensor.matmul(
                                    pb[:, ds(nn * 512, 512)],
                                    xnTl[b][:, dc, ts(cc, 128)],
                                    wqk_t[:, dc, ds(nn * 512, 512)],
                                    start=(dc == 0),
                                    stop=(dc == NCH - 1 and not has_qkb))
                        if has_qkb:
                            nc.tensor.matmul(pb[:, 0:HW], ones_t, qkb_t,
                                             start=False, stop=True)
                        eng = nc.scalar if cc == 0 else nc.vector
                        _copy(eng, ql[b][:, cc, :], pb)

                def stageDk(b):
                    # kT = wqk_k.T @ xnT  [hw, c] (no transpose needed)
                    for jh in range(2):
                        pb = psB.tile([128, HW], f32, tag="pb", name="psk")
                        for jc in range(4):
                            j = jh * 4 + jc
                            for dc in range(NCH):
                                nc.tensor.matmul(
                                    pb[:, ds(jc * 256, 256)],
                                    wqk_t[:, dc, ds(HW + j * 128, 128)],
                                    xnTl[b][:, dc, :],
                                    start=(dc == 0), stop=(dc == NCH - 1))
                        if has_qkb:
                            for jc in range(4):
                                j = jh * 4 + jc
                                nc.scalar.activation(
                                    out=kTl[b][:, j, :],
                                    in_=pb[:, ds(jc * 256, 256)],
                                    func=mybir.ActivationFunctionType.Identity,
                                    bias=qkbk_t[:, j:j + 1], scale=1.0)
                        else:
                            for jc in range(4):
                                j = jh * 4 + jc
                                eng = nc.scalar if jc % 2 == 0 else nc.vector
                                _copy(eng, kTl[b][:, j, :],
                                      pb[:, ds(jc * 256, 256)])

                def stageE(b, br):
                    # kw = k @ (scale*w_b.T)  [c, hw]
                    wsd = wspa_t if br == 0 else wfrq_t
                    for cc in range(CC):
                        pb = psB.tile([128, HW], f32, tag="pb", name="pse")
                        for mc in range(NCH):
                            for jj in range(2):
                                nc.tensor.matmul(
                                    pb[:, ds(jj * 512, 512)],
                                    kTl[b][:, mc, ts(cc, 128)],
                                    wsd[:, mc, ds(jj * 512, 512)],
                                    start=(mc == 0), stop=(mc == NCH - 1))
                        eng = nc.vector if cc == 0 else nc.scalar
                        _copy(eng, kwl[br][:, cc, :], pb)

                def stageF(b, br, ets, vtns):
                    # logits -> exp(+rowsum) -> et, vtn
                    lb_t = bspa_t if br == 0 else bfrq_t
                    vt = vtsl[b] if br == 0 else vtfl[b]
                    for nk in range(NCH):
                        pl = psB.tile([128, HW], f32, tag="pb", name="pl")
                        for cc in range(CC):
                            for jj in range(2):
                                nc.tensor.matmul(
                                    pl[:, ds(jj * 512, 512)],
                                    ql[b][:, cc, ts(nk, 128)],
                                    kwl[br][:, cc, ds(jj * 512, 512)],
                                    start=(cc == 0),
                                    stop=(cc == CC - 1 and lb_t is None))
                        if lb_t is not None:
                            for jj in range(2):
                                nc.tensor.matmul(
                                    pl[:, ds(jj * 512, 512)], ones_t,
                                    lb_t[:, ds(jj * 512, 512)],
                                    start=False, stop=True)
                        et = attp.tile([128, HW], bf16, tag="att",
                                       name=f"et{b}_{br}_{nk}")
                        rsum = small.tile([128, 1], f32, tag="rs", name="rsum")
                        nc.scalar.activation(out=et, in_=pl, func=Exp,
                                             accum_out=rsum)
                        rrec = small.tile([128, 1], f32, tag="rr", name="rrec")
                        nc.vector.reciprocal(out=rrec, in_=rsum)
                        vtn = attp.tile([128, C], bf16, tag="vtn", name="vtn")
                        nc.vector.tensor_scalar_mul(out=vtn, in0=vt[:, nk, :],
                                                    scalar1=rrec)
                        ets.append(et)
                        vtns.append(vtn)

                def stageG(b, br, ets, vtns):
                    # out_b = x_b + vT.T @ att  (residual add split DVE/Pool)
                    out_d = os_d if br == 0 else of_d
                    x_res = xsl[b] if br == 0 else xfl[b]
                    for cc in range(CC):
                        pg = psB.tile([128, HW], f32, tag="pb", name="pg")
                        for nk in range(NCH):
                            for jj in range(2):
                                nc.tensor.matmul(
                                    pg[:, ds(jj * 512, 512)],
                                    vtns[nk][:, ts(cc, 128)],
                                    ets[nk][:, ds(jj * 512, 512)],
                                    start=(nk == 0), stop=(nk == NCH - 1))
                        res = resp.tile([128, HW], f32, tag="res",
                                        name=f"res{b}_{br}_{cc}")
                        for hh in range(2):
                            nc.vector.tensor_tensor(
                                out=res[:, ds(hh * 512, 512)],
                                in0=pg[:, ds(hh * 512, 512)],
                                in1=x_res[:, cc, ds(hh * 512, 512)]
                                .bitcast(f32), op=ADD)
                            qeng = nc.scalar if (cc + hh) % 2 == 0 else nc.sync
                            qeng.dma_start(
                                out=out_d[b, ds(cc * 128, 128),
                                          ds(hh * 512, 512)],
                                in_=res[:, ds(hh * 512, 512)])

                # ---- schedule ----
                stageA(0); stageB(0); stageLN(0)
                stageA(1); stageB(1); stageLN(1)
                stageC(0); stageDq(0); stageDk(0)
                e0 = ([], []); e1 = ([], [])
                stageE(0, 0); stageF(0, 0, *e0)
                stageE(0, 1)
                stageG(0, 0, *e0); stageF(0, 1, *e1)
                stageC(1); stageDq(1); stageDk(1)
                stageG(0, 1, *e1)
                e2 = ([], []); e3 = ([], [])
                stageE(1, 0); stageF(1, 0, *e2)
                stageE(1, 1)
                stageG(1, 0, *e2); stageF(1, 1, *e3)
                stageG(1, 1, *e3)

            if reps == 1:
                _samples_body()
            elif isinstance(reps, tuple):      # ("unroll", R)
                for _rep in range(reps[1]):
                    _samples_body()
            else:
                with tc.For_i(0, reps, 1):
                    _samples_body()

    nc.compile()
    return nc


def _prep_base(w_cdc, w_sv, w_fv, ln_w, ln_b, w_qk, w_spa, b_spa,
               w_frq, b_frq, b_sv, b_fv):
    """Host-side weight prep shared by kernel() and the bench harness."""
    scale = float(HW) ** -0.5
    qkb = np.asarray(ln_b, np.float32) @ np.asarray(w_qk, np.float32).T
    flags = (bool(np.any(qkb)), bool(np.any(b_spa)), bool(np.any(b_frq)),
             bool(np.any(b_sv)), bool(np.any(b_fv)))
    base = {
        "wcdcT": _round_f32r(np.asarray(w_cdc, np.float32).T),
        "wsvT": _round_f32r(np.asarray(w_sv, np.float32).T),
        "wfvT": _round_f32r(np.asarray(w_fv, np.float32).T),
        "wqkTg": _bf16(np.asarray(w_qk, np.float32).T
                       * np.asarray(ln_w, np.float32)[:, None]),
        "wspaT": _bf16(np.asarray(w_spa, np.float32).T * scale),
        "wfrqT": _bf16(np.asarray(w_frq, np.float32).T * scale),
    }
    if flags[0]:
        base["qkb"] = _round_f32r(qkb[None, :HW])
        base["qkbk"] = np.ascontiguousarray(
            qkb[HW:].reshape(NCH, 128).T, np.float32)
    if flags[1]:
        base["bspa"] = _round_f32r(np.asarray(b_spa, np.float32)[None, :])
    if flags[2]:
        base["bfrq"] = _round_f32r(np.asarray(b_frq, np.float32)[None, :])
    if flags[3]:
        base["bsv"] = _round_f32r(np.asarray(b_sv, np.float32)[None, :])
    if flags[4]:
        base["bfv"] = _round_f32r(np.asarray(b_fv, np.float32)[None, :])
    return base, flags


def kernel(x_spa, x_freq, w_cdc, b_cdc, w_sv, b_sv, w_fv, b_fv,
           ln_w, ln_b, w_qk, w_spa, b_spa, w_frq, b_frq):
    # b_cdc is a per-row constant added before LayerNorm over that row: no-op.
    base, flags = _prep_base(w_cdc, w_sv, w_fv, ln_w, ln_b, w_qk,
                             w_spa, b_spa, w_frq, b_frq, b_sv, b_fv)
    if flags not in _CACHE:
        _CACHE[flags] = _build(flags)
    nc = _CACHE[flags]

    xs = _round_f32r(np.asarray(x_spa, np.float32).reshape(B, C, HW))
    xf = _round_f32r(np.asarray(x_freq, np.float32).reshape(B, C, HW))
    in_maps = []
    for c in range(NCORES):
        m = dict(base)
        m["xs"] = xs[c * BPC:(c + 1) * BPC]
        m["xf"] = xf[c * BPC:(c + 1) * BPC]
        in_maps.append(m)

    res = bass_utils.run_bass_kernel_spmd(nc, in_maps,
                                          core_ids=list(range(NCORES)))
    out_spa = np.concatenate([res.results[c]["os"] for c in range(NCORES)],
                             axis=0)
    out_frq = np.concatenate([res.results[c]["of"] for c in range(NCORES)],
                             axis=0)
    return (out_spa.reshape(B, C, H, W).astype(np.float32),
            out_frq.reshape(B, C, H, W).astype(np.float32))


# revision 22
# speedup vs baseline: 2.8104x; 2.8104x over previous
"""Trainium2 Bass kernel for nn_CMIA_2843268350555 (dual-branch spatial/freq attention).

Strategy: data-parallel over batch (16 samples / 8 cores = 2 per core).
All matmul operands are float32r: any 16-bit matmul operand makes the
legalizer emit a standalone (non-overlapped, walrus ldw-opt disabled)
InstLdweights per matmul, which costs far more on HW than the dtype saves.

Per-sample math (C=256 channels, HW=1024):
  vT_b    = (x_b.T @ w_bv.T)            [hw, c]   (b in {spa, frq})
  x       = w_cdc @ [x_spa; x_frq]      [c, hw]   (+b_cdc: no-op through LN)
  xn      = layernorm_rows(x)           [c, hw]   (affine folded into wqkTg)
  xnT     = transpose(xn)               [hw, c]
  q,k     = xn @ wqkTg                  [c, hw] each
  kT      = k.T                         [hw, c]
  kw_b    = (kT.T @ (scale*w_b.T))      [c, hw]
  logits  = q.T @ kw_b                  [hw(n), hw(j)]
  att_b   = softmax_j(logits + b_b)     (1/rowsum folded into vT)
  out_b   = x_b + (vT_b.T @ att_b)      [c, hw]

Schedule (per 2-sample iteration), interleaved to keep PE dense:
  A0 B0 LN0 A1 B1 LN1 C0 D0 KT0 | brs(0) brf(0) | C1 D1 KT1 | brs(1) brf(1)
Each branch: E (kw, 2 big psum tiles), then F/G software-pipelined per nk
(G(nk-1) emitted between F(nk) and its softmax chain so exp latency hides).
PSUM: psS 2x[128,512] (A/B/C/D/KT) + psL 2x[128,512] (logit halves) +
psBG 2x[128,1024] (E accum / G accum) = 8 banks.
LayerNorm rstd is computed on DVE only (reciprocal-seeded Newton): the Act
Sqrt would thrash activation-function tables against the softmax Exp.
"""
import numpy as np

import concourse.bacc as bacc
import concourse.mybir as mybir
import concourse.tile as tile
from concourse import bass_utils
from concourse.bass import ts, ds
from concourse.masks import make_identity

f32 = mybir.dt.float32
f32r = mybir.dt.float32r
bf16 = mybir.dt.bfloat16

B, C, H, W = 16, 256, 32, 32
HW = H * W           # 1024
J2 = 2 * HW          # 2048
NCORES = 8
BPC = B // NCORES    # samples per core
CC = C // 128        # 2 channel chunks
NCH = HW // 128      # 8 hw chunks
EPS = 1e-5


def _round_f32r(x: np.ndarray) -> np.ndarray:
    """RNE-round fp32 to fp32r (11 mantissa bits; low 12 bits zero)."""
    x = np.ascontiguousarray(x, dtype=np.float32)
    u = x.view(np.uint32)
    lsb = (u >> np.uint32(12)) & np.uint32(1)
    r = u + np.uint32(0x7FF) + lsb
    return (r & ~np.uint32(0xFFF)).view(np.float32)


_CACHE: dict = {}


def _build(flags, reps=1):
    has_qkb, has_bspa, has_bfrq, has_bsv, has_bfv = flags
    any_mm_bias = has_qkb or has_bspa or has_bfrq or has_bsv or has_bfv

    nc = bacc.Bacc("TRN2", target_bir_lowering=False, debug=False,
                   enable_asserts=True, num_devices=NCORES)
    xs_d = nc.dram_tensor("xs", [BPC, C, HW], f32r, kind="ExternalInput").ap()
    xf_d = nc.dram_tensor("xf", [BPC, C, HW], f32r, kind="ExternalInput").ap()
    wcdc_d = nc.dram_tensor("wcdcT", [2 * C, C], f32r, kind="ExternalInput").ap()
    wsv_d = nc.dram_tensor("wsvT", [C, C], f32r, kind="ExternalInput").ap()
    wfv_d = nc.dram_tensor("wfvT", [C, C], f32r, kind="ExternalInput").ap()
    wqk_d = nc.dram_tensor("wqkTg", [HW, J2], f32r, kind="ExternalInput").ap()
    wspa_d = nc.dram_tensor("wspaT", [HW, HW], f32r, kind="ExternalInput").ap()
    wfrq_d = nc.dram_tensor("wfrqT", [HW, HW], f32r, kind="ExternalInput").ap()
    qkb_d = qkbk_d = bspa_d = bfrq_d = bsv_d = bfv_d = None
    if has_qkb:
        qkb_d = nc.dram_tensor("qkb", [1, HW], f32r, kind="ExternalInput").ap()
        qkbk_d = nc.dram_tensor("qkbk", [1, HW], f32r,
                                kind="ExternalInput").ap()
    if has_bspa:
        bspa_d = nc.dram_tensor("bspa", [1, HW], f32r, kind="ExternalInput").ap()
    if has_bfrq:
        bfrq_d = nc.dram_tensor("bfrq", [1, HW], f32r, kind="ExternalInput").ap()
    if has_bsv:
        bsv_d = nc.dram_tensor("bsv", [1, C], f32r, kind="ExternalInput").ap()
    if has_bfv:
        bfv_d = nc.dram_tensor("bfv", [1, C], f32r, kind="ExternalInput").ap()
    os_d = nc.dram_tensor("os", [BPC, C, HW], f32, kind="ExternalOutput").ap()
    of_d = nc.dram_tensor("of", [BPC, C, HW], f32, kind="ExternalOutput").ap()

    Exp = mybir.ActivationFunctionType.Exp
    SUB = mybir.AluOpType.subtract
    MUL = mybir.AluOpType.mult
    ADD = mybir.AluOpType.add

    with tile.TileContext(nc) as tc:
        with tc.tile_pool(name="constp", bufs=1) as constp, \
             tc.tile_pool(name="wqkp", bufs=1) as wqkp, \
             tc.tile_pool(name="wsp", bufs=4) as wsp, \
             tc.tile_pool(name="xin", bufs=2) as xin, \
             tc.tile_pool(name="dat", bufs=2) as dat, \
             tc.tile_pool(name="attp", bufs=4) as attp, \
             tc.tile_pool(name="resp", bufs=2) as resp, \
             tc.tile_pool(name="small", bufs=4) as small, \
             tc.tile_pool(name="psS", bufs=2, space="PSUM") as psS, \
             tc.tile_pool(name="psL", bufs=2, space="PSUM") as psL, \
             tc.tile_pool(name="psBG", bufs=2, space="PSUM") as psBG:

            # ---- resident weights ----
            # Queues: ACT = small weights (fast HWDGE, land first) + sample-1
            # inputs + output stores; Pool(SWDGE) = wqk burst (its SEQ has no
            # early compute to clog); SP = sample-0 inputs + streamed ws.
            wsv_t = constp.tile([128, CC, C], f32r, name="wsv_t")
            nc.scalar.dma_start(out=wsv_t,
                                in_=wsv_d.rearrange("(kc p) c -> p kc c", p=128))
            wfv_t = constp.tile([128, CC, C], f32r, name="wfv_t")
            nc.scalar.dma_start(out=wfv_t,
                                in_=wfv_d.rearrange("(kc p) c -> p kc c", p=128))
            wcdc_t = constp.tile([128, 4, C], f32r, name="wcdc_t")
            nc.scalar.dma_start(out=wcdc_t,
                                in_=wcdc_d.rearrange("(kc p) c -> p kc c", p=128))
            ident_f = constp.tile([128, 128], f32, name="ident_f")
            make_identity(nc, ident_f)
            ones_t = None
            if any_mm_bias:
                ones_f = constp.tile([1, 128], f32, name="ones_f")
                nc.vector.memset(ones_f, 1.0)
                ones_t = constp.tile([1, 128], f32r, name="ones_t")
                nc.scalar.copy(out=ones_t, in_=ones_f)

            def _bias_tile(dram, n, nm):
                t = constp.tile([1, n], f32r, name=nm)
                nc.gpsimd.dma_start(out=t, in_=dram)
                return t

            qkb_t = _bias_tile(qkb_d, HW, "qkb_t") if has_qkb else None
            qkbk_t = _bias_tile(qkbk_d, HW, "qkbk_t") if has_qkb else None
            bspa_t = _bias_tile(bspa_d, HW, "bspa_t") if has_bspa else None
            bfrq_t = _bias_tile(bfrq_d, HW, "bfrq_t") if has_bfrq else None
            bsv_t = _bias_tile(bsv_d, C, "bsv_t") if has_bsv else None
            bfv_t = _bias_tile(bfv_d, C, "bfv_t") if has_bfv else None

            def _copy(eng, out, in_):
                if eng is nc.scalar:
                    eng.copy(out=out, in_=in_)
                else:
                    eng.tensor_copy(out=out, in_=in_)

            wqk_t = wqkp.tile([128, NCH, J2], f32r, name="wqk_t")
            for kc in range(NCH):
                nc.gpsimd.dma_start(out=wqk_t[:, kc, :],
                                    in_=wqk_d[ds(kc * 128, 128), :])

            def _samples_body():
                xsl, xfl, vtsl, vtfl, xsbl, xnTl, ql, kTl = \
                    [], [], [], [], [], [], [], []
                for b in range(BPC):
                    # sample 0 inputs on SP, sample 1 on ACT: two queues race
                    # ahead instead of one serial stream behind the weights
                    ieng = nc.sync if b == 0 else nc.scalar
                    xs_t = xin.tile([128, CC, HW], f32r, tag="xs", name=f"xs{b}")
                    ieng.dma_start(
                        out=xs_t,
                        in_=xs_d[b].rearrange("(cc p) n -> p cc n", p=128))
                    xf_t = xin.tile([128, CC, HW], f32r, tag="xf", name=f"xf{b}")
                    ieng.dma_start(
                        out=xf_t,
                        in_=xf_d[b].rearrange("(cc p) n -> p cc n", p=128))
                    xsl.append(xs_t)
                    xfl.append(xf_t)
                    vtsl.append(dat.tile([128, NCH, C], bf16, tag="vts",
                                         name=f"vts{b}"))
                    vtfl.append(dat.tile([128, NCH, C], bf16, tag="vtf",
                                         name=f"vtf{b}"))
                    # x_sb and (later) k_sb share the "xc" ring: x_sb{b} dies
                    # at C{b} before k_sb{b} is born at D{b}
                    xsbl.append(dat.tile([128, CC, HW], f32, tag="xc",
                                         name=f"xsb{b}"))
                    xnTl.append(dat.tile([128, NCH, C], f32r, tag="xnT",
                                         bufs=1, name=f"xnT{b}"))
                    ql.append(dat.tile([128, CC, HW], f32r, tag="q",
                                       bufs=1, name=f"q{b}"))
                    kTl.append(dat.tile([128, NCH, C], f32r, tag="kT",
                                        bufs=1, name=f"kT{b}"))
                kwl = [dat.tile([128, CC, HW], f32r, tag=f"kw{br}", bufs=1,
                                name=f"kw{br}") for br in range(2)]

                def stageA(b):
                    # value projections vT_b = x_b.T @ w_bv.T  [hw, c]
                    # two mc chunks share one PSUM tile -> one grouped copy
                    for i, (src, wv, dst, bt) in enumerate(
                            ((xsl[b], wsv_t, vtsl[b], bsv_t),
                             (xfl[b], wfv_t, vtfl[b], bfv_t))):
                        for mg in range(NCH // 2):
                            ps = psS.tile([128, 512], f32, tag="ps", name="psa")
                            for half in range(2):
                                mc = mg * 2 + half
                                for kc in range(CC):
                                    nc.tensor.matmul(
                                        ps[:, ds(half * C, C)],
                                        src[:, kc, ts(mc, 128)], wv[:, kc, :],
                                        start=(kc == 0),
                                        stop=(kc == CC - 1 and bt is None))
                                if bt is not None:
                                    nc.tensor.matmul(ps[:, ds(half * C, C)],
                                                     ones_t, bt,
                                                     start=False, stop=True)
                            eng = nc.vector if mg % 2 == 0 else nc.scalar
                            _copy(eng, dst[:, ds(mg * 2, 2), :],
                                  ps.rearrange("p (a f) -> p a f", a=2))

                def stageB(b):
                    # x = w_cdc @ [xs; xf]  [c, hw]
                    for cc in range(CC):
                        for nn in range(2):
                            ps = psS.tile([128, 512], f32, tag="ps", name="psb")
                            for kc in range(4):
                                src = xsl[b] if kc < 2 else xfl[b]
                                nc.tensor.matmul(
                                    ps, wcdc_t[:, kc, ts(cc, 128)],
                                    src[:, kc % 2, ds(nn * 512, 512)],
                                    start=(kc == 0), stop=(kc == 3))
                            eng = nc.scalar if nn == 0 else nc.vector
                            _copy(eng, xsbl[b][:, cc, ds(nn * 512, 512)], ps)

                def stageLN(b):
                    # rstd = 1/sqrt(var+eps) on DVE only: Newton iteration
                    # y' = y*(1.5 - 0.5*v*y^2), seeded y0 = min(1/v, 1) so
                    # v*y0^2 <= 1 < 3 (convergent for any v > 0). Avoids the
                    # Act-engine Sqrt table load that thrashes against Exp.
                    x_sb = xsbl[b]
                    mvs = []
                    for cc in range(CC):
                        xr = x_sb[:, cc, :].rearrange("p (s f) -> p s f", s=2)
                        stats = small.tile([128, 2, 6], f32, tag="st",
                                           name="stats")
                        for s in range(2):
                            nc.vector.bn_stats(out=stats[:, s, :],
                                               in_=xr[:, s, :])
                        mv = small.tile([128, 2], f32, tag=f"mv{cc}",
                                        name="mv")
                        nc.vector.bn_aggr(out=mv, in_=stats)
                        mvs.append(mv)
                    veps = small.tile([128, 2], f32, tag="veps", name="veps")
                    for cc in range(CC):
                        nc.vector.tensor_scalar(
                            out=veps[:, cc:cc + 1], in0=mvs[cc][:, 1:2],
                            scalar1=EPS, scalar2=None, op0=ADD)
                    y = small.tile([128, 2], f32, tag="rstd", name="rstd")
                    nc.vector.reciprocal(out=y, in_=veps)
                    nc.vector.tensor_scalar_min(out=y, in0=y, scalar1=1.0)
                    t = small.tile([128, 2], f32, tag="nt", name="nt")
                    for _ in range(5):
                        nc.vector.tensor_tensor(out=t, in0=y, in1=y, op=MUL)
                        nc.vector.tensor_tensor(out=t, in0=t, in1=veps,
                                                op=MUL)
                        nc.vector.tensor_scalar(
                            out=t, in0=t, scalar1=-0.5, scalar2=1.5,
                            op0=MUL, op1=ADD)
                        nc.vector.tensor_tensor(out=y, in0=y, in1=t, op=MUL)
                    for cc in range(CC):
                        nc.vector.tensor_scalar(
                            out=x_sb[:, cc, :], in0=x_sb[:, cc, :],
                            scalar1=mvs[cc][:, 0:1], scalar2=y[:, cc:cc + 1],
                            op0=SUB, op1=MUL)

                def stageC(b):
                    # xnT = xn.T: 4 transposes per PSUM tile, 1 grouped copy
                    for cc in range(CC):
                        for dg in range(2):
                            pt = psS.tile([128, 512], f32, tag="ps", name="pt")
                            for i in range(4):
                                dc = dg * 4 + i
                                nc.tensor.transpose(
                                    pt[:, ds(i * 128, 128)],
                                    xsbl[b][:, cc, ds(dc * 128, 128)],
                                    ident_f)
                            eng = nc.scalar if dg == 0 else nc.vector
                            _copy(eng,
                                  xnTl[b][:, ds(dg * 4, 4), ts(cc, 128)],
                                  pt.rearrange("p (a f) -> p a f", a=4))

                def stageD(b, k_sb):
                    # q,k = xn @ wqkTg (f32r fused weight loads are free, so
                    # the jj-inner stationary reload pattern costs nothing)
                    for cc in range(CC):
                        for half in range(2):
                            for jj in range(2):
                                ps = psS.tile([128, 512], f32, tag="ps",
                                              name="psd")
                                for dc in range(NCH):
                                    nc.tensor.matmul(
                                        ps, xnTl[b][:, dc, ts(cc, 128)],
                                        wqk_t[:, dc,
                                              ds(half * HW + jj * 512, 512)],
                                        start=(dc == 0),
                                        stop=(dc == NCH - 1 and not has_qkb))
                                if has_qkb:
                                    bt = qkb_t if half == 0 else qkbk_t
                                    nc.tensor.matmul(
                                        ps, ones_t, bt[:, ds(jj * 512, 512)],
                                        start=False, stop=True)
                                eng = nc.scalar if jj == 0 else nc.vector
                                tgt = ql[b] if half == 0 else k_sb
                                _copy(eng, tgt[:, cc, ds(jj * 512, 512)], ps)

                def stageKT(b, k_sb):
                    # kT = k.T via PE transposes (f32r fused loads)
                    for cc in range(CC):
                        for dg in range(2):
                            pt = psS.tile([128, 512], f32, tag="ps", name="ptk")
                            for i in range(4):
                                mc = dg * 4 + i
                                nc.tensor.transpose(
                                    pt[:, ds(i * 128, 128)],
                                    k_sb[:, cc, ds(mc * 128, 128)], ident_f)
                            eng = nc.scalar if dg == 0 else nc.vector
                            _copy(eng,
                                  kTl[b][:, ds(dg * 4, 4), ts(cc, 128)],
                                  pt.rearrange("p (a f) -> p a f", a=4))

                def branch(b, br):
                    wsd = wspa_d if br == 0 else wfrq_d
                    lb_t = bspa_t if br == 0 else bfrq_t
                    vt = vtsl[b] if br == 0 else vtfl[b]
                    out_d = os_d if br == 0 else of_d
                    x_res = xsl[b] if br == 0 else xfl[b]
                    kw = kwl[br]

                    # E: kw = k @ (scale*w_b.T); ws streamed from HBM on SP,
                    # mc-outer so each chunk is consumed as it lands
                    ws_tiles = []
                    for mc in range(NCH):
                        wst = wsp.tile([128, HW], f32r, tag="ws",
                                       name=f"ws{b}_{br}_{mc}")
                        nc.sync.dma_start(out=wst,
                                          in_=wsd[ds(mc * 128, 128), :])
                        ws_tiles.append(wst)
                    pse = [psBG.tile([128, HW], f32, tag="bg",
                                     name=f"pse{b}_{br}_{cc}")
                           for cc in range(CC)]
                    for mc in range(NCH):
                        for cc in range(CC):
                            for jj in range(2):
                                nc.tensor.matmul(
                                    pse[cc][:, ds(jj * 512, 512)],
                                    kTl[b][:, mc, ts(cc, 128)],
                                    ws_tiles[mc][:, ds(jj * 512, 512)],
                                    start=(mc == 0), stop=(mc == NCH - 1))
                    for cc in range(CC):
                        eng = nc.vector if cc == 0 else nc.scalar
                        _copy(eng, kw[:, cc, :], pse[cc])

                    # F/G pipelined per nk: logit halves -> exp(+rowsum
                    # halves) -> vtn; G(nk-1) emitted after F(nk) so the
                    # softmax chain of nk hides under G(nk-1)'s matmuls
                    psg = [psBG.tile([128, HW], f32, tag="bg",
                                     name=f"psg{b}_{br}_{cc}")
                           for cc in range(CC)]
                    ets, vtns = [], []

                    def emit_F(nk):
                        rs = small.tile([128, 2], f32, tag="rs", name="rsum")
                        pair = []
                        for hh in range(2):
                            pl = psL.tile([128, 512], f32, tag="pl", name="pl")
                            for cc in range(CC):
                                nc.tensor.matmul(
                                    pl, ql[b][:, cc, ts(nk, 128)],
                                    kw[:, cc, ds(hh * 512, 512)],
                                    start=(cc == 0),
                                    stop=(cc == CC - 1 and lb_t is None))
                            if lb_t is not None:
                                nc.tensor.matmul(pl, ones_t,
                                                 lb_t[:, ds(hh * 512, 512)],
                                                 start=False, stop=True)
                            et = attp.tile([128, 512], f32r, tag="att",
                                           name=f"et{b}_{br}_{nk}_{hh}")
                            nc.scalar.activation(out=et, in_=pl, func=Exp,
                                                 accum_out=rs[:, hh:hh + 1])
                            pair.append(et)
                        ets.append(pair)
                        rtot = small.tile([128, 1], f32, tag="rt", name="rtot")
                        nc.vector.tensor_tensor(out=rtot, in0=rs[:, 0:1],
                                                in1=rs[:, 1:2], op=ADD)
                        nc.vector.reciprocal(out=rtot, in_=rtot)
                        vtn = attp.tile([128, C], f32r, tag="vtn", bufs=2,
                                        name="vtn")
                        nc.vector.tensor_scalar_mul(out=vtn, in0=vt[:, nk, :],
                                                    scalar1=rtot)
                        vtns.append(vtn)

                    def emit_G(nk):
                        for cc in range(CC):
                            for hh in range(2):
                                nc.tensor.matmul(
                                    psg[cc][:, ds(hh * 512, 512)],
                                    vtns[nk][:, ts(cc, 128)], ets[nk][hh],
                                    start=(nk == 0), stop=(nk == NCH - 1))

                    for nk in range(NCH):
                        emit_F(nk)
                        if nk > 0:
                            emit_G(nk - 1)
                    emit_G(NCH - 1)

                    # residual add + store, split in halves for early outflow
                    for cc in range(CC):
                        for hh in range(2):
                            res = resp.tile([128, 512], f32, tag="res",
                                            name=f"res{b}_{br}_{cc}_{hh}")
                            nc.vector.tensor_tensor(
                                out=res, in0=psg[cc][:, ds(hh * 512, 512)],
                                in1=x_res[:, cc, ds(hh * 512, 512)]
                                .bitcast(f32), op=ADD)
                            nc.scalar.dma_start(
                                out=out_d[b, ds(cc * 128, 128),
                                          ds(hh * 512, 512)],
                                in_=res)

                # ---- schedule ----
                stageA(0); stageB(0); stageLN(0)
                stageA(1); stageB(1); stageLN(1)
                stageC(0)
                k_sb0 = dat.tile([128, CC, HW], f32, tag="xc", name="ksb0")
                stageD(0, k_sb0); stageKT(0, k_sb0)
                branch(0, 0)
                branch(0, 1)
                stageC(1)
                k_sb1 = dat.tile([128, CC, HW], f32, tag="xc", name="ksb1")
                stageD(1, k_sb1); stageKT(1, k_sb1)
                branch(1, 0)
                branch(1, 1)

            if reps == 1:
                _samples_body()
            elif isinstance(reps, tuple):      # ("unroll", R)
                for _rep in range(reps[1]):
                    _samples_body()
            else:
                with tc.For_i(0, reps, 1):
                    _samples_body()

    nc.compile()
    return nc


def _prep_base(w_cdc, w_sv, w_fv, ln_w, ln_b, w_qk, w_spa, b_spa,
               w_frq, b_frq, b_sv, b_fv):
    """Host-side weight prep shared by kernel() and the bench harness."""
    scale = float(HW) ** -0.5
    qkb = np.asarray(ln_b, np.float32) @ np.asarray(w_qk, np.float32).T
    flags = (bool(np.any(qkb)), bool(np.any(b_spa)), bool(np.any(b_frq)),
             bool(np.any(b_sv)), bool(np.any(b_fv)))
    base = {
        "wcdcT": _round_f32r(np.asarray(w_cdc, np.float32).T),
        "wsvT": _round_f32r(np.asarray(w_sv, np.float32).T),
        "wfvT": _round_f32r(np.asarray(w_fv, np.float32).T),
        "wqkTg": _round_f32r(np.asarray(w_qk, np.float32).T
                             * np.asarray(ln_w, np.float32)[:, None]),
        "wspaT": _round_f32r(np.asarray(w_spa, np.float32).T * scale),
        "wfrqT": _round_f32r(np.asarray(w_frq, np.float32).T * scale),
    }
    if flags[0]:
        base["qkb"] = _round_f32r(qkb[None, :HW])
        base["qkbk"] = _round_f32r(qkb[None, HW:])
    if flags[1]:
        base["bspa"] = _round_f32r(np.asarray(b_spa, np.float32)[None, :])
    if flags[2]:
        base["bfrq"] = _round_f32r(np.asarray(b_frq, np.float32)[None, :])
    if flags[3]:
        base["bsv"] = _round_f32r(np.asarray(b_sv, np.float32)[None, :])
    if flags[4]:
        base["bfv"] = _round_f32r(np.asarray(b_fv, np.float32)[None, :])
    return base, flags


def kernel(x_spa, x_freq, w_cdc, b_cdc, w_sv, b_sv, w_fv, b_fv,
           ln_w, ln_b, w_qk, w_spa, b_spa, w_frq, b_frq):
    # b_cdc is a per-row constant added before LayerNorm over that row: no-op.
    base, flags = _prep_base(w_cdc, w_sv, w_fv, ln_w, ln_b, w_qk,
                             w_spa, b_spa, w_frq, b_frq, b_sv, b_fv)
    if flags not in _CACHE:
        _CACHE[flags] = _build(flags)
    nc = _CACHE[flags]

    xs = _round_f32r(np.asarray(x_spa, np.float32).reshape(B, C, HW))
    xf = _round_f32r(np.asarray(x_freq, np.float32).reshape(B, C, HW))
    in_maps = []
    for c in range(NCORES):
        m = dict(base)
        m["xs"] = xs[c * BPC:(c + 1) * BPC]
        m["xf"] = xf[c * BPC:(c + 1) * BPC]
        in_maps.append(m)

    res = bass_utils.run_bass_kernel_spmd(nc, in_maps,
                                          core_ids=list(range(NCORES)))
    out_spa = np.concatenate([res.results[c]["os"] for c in range(NCORES)],
                             axis=0)
    out_frq = np.concatenate([res.results[c]["of"] for c in range(NCORES)],
                             axis=0)
    return (out_spa.reshape(B, C, H, W).astype(np.float32),
            out_frq.reshape(B, C, H, W).astype(np.float32))


# revision 23
# speedup vs baseline: 4.3303x; 1.5408x over previous
"""Trainium2 Bass kernel for nn_CMIA_2843268350555 (dual-branch spatial/freq attention).

Strategy: data-parallel over batch (16 samples / 8 cores = 2 per core).
All matmul operands are float32r: any 16-bit matmul operand makes the
legalizer emit a standalone (non-overlapped, walrus ldw-opt disabled)
InstLdweights per matmul, which costs far more on HW than the dtype saves.

Per-sample math (C=256 channels, HW=1024):
  vT_b    = (x_b.T @ w_bv.T)            [hw, c]   (b in {spa, frq})
  x       = w_cdc @ [x_spa; x_frq]      [c, hw]   (+b_cdc: no-op through LN)
  xn      = layernorm_rows(x)           [c, hw]   (affine folded into wqkTg)
  xnT     = transpose(xn)               [hw, c]
  q,k     = xn @ wqkTg                  [c, hw] each
  kT      = k.T                         [hw, c]
  kw_b    = (kT.T @ (scale*w_b.T))      [c, hw]
  logits  = q.T @ kw_b                  [hw(n), hw(j)]
  att_b   = softmax_j(logits + b_b)     (1/rowsum folded into vT)
  out_b   = x_b + (vT_b.T @ att_b)      [c, hw]

Schedule (per 2-sample iteration), interleaved to keep PE dense:
  A0 B0 LN0 A1 B1 LN1 C0 D0 KT0 | brs(0) brf(0) | C1 D1 KT1 | brs(1) brf(1)
Each branch: E (kw, 2 big psum tiles), then F/G software-pipelined per nk
(G(nk-1) emitted between F(nk) and its softmax chain so exp latency hides).
PSUM: psS 2x[128,512] (A/B/C/D/KT) + psL 2x[128,512] (logit halves) +
psBG 2x[128,1024] (E accum / G accum) = 8 banks.
LayerNorm rstd is computed on DVE only (reciprocal-seeded Newton): the Act
Sqrt would thrash activation-function tables against the softmax Exp.
"""
import numpy as np

import concourse.bacc as bacc
import concourse.mybir as mybir
import concourse.tile as tile
from concourse import bass_utils
from concourse.bass import ts, ds
from concourse.masks import make_identity

f32 = mybir.dt.float32
f32r = mybir.dt.float32r
bf16 = mybir.dt.bfloat16

B, C, H, W = 16, 256, 32, 32
HW = H * W           # 1024
J2 = 2 * HW          # 2048
NCORES = 8
BPC = B // NCORES    # samples per core
CC = C // 128        # 2 channel chunks
NCH = HW // 128      # 8 hw chunks
EPS = 1e-5


def _round_f32r(x: np.ndarray) -> np.ndarray:
    """RNE-round fp32 to fp32r (11 mantissa bits; low 12 bits zero)."""
    x = np.ascontiguousarray(x, dtype=np.float32)
    u = x.view(np.uint32)
    lsb = (u >> np.uint32(12)) & np.uint32(1)
    r = u + np.uint32(0x7FF) + lsb
    return (r & ~np.uint32(0xFFF)).view(np.float32)


_CACHE: dict = {}


def _build(flags, reps=1):
    has_qkb, has_bspa, has_bfrq, has_bsv, has_bfv = flags
    any_mm_bias = has_qkb or has_bspa or has_bfrq or has_bsv or has_bfv

    nc = bacc.Bacc("TRN2", target_bir_lowering=False, debug=False,
                   enable_asserts=True, num_devices=NCORES)
    xs_d = nc.dram_tensor("xs", [BPC, C, HW], f32r, kind="ExternalInput").ap()
    xf_d = nc.dram_tensor("xf", [BPC, C, HW], f32r, kind="ExternalInput").ap()
    wcdc_d = nc.dram_tensor("wcdcT", [2 * C, C], f32r, kind="ExternalInput").ap()
    wsv_d = nc.dram_tensor("wsvT", [C, C], f32r, kind="ExternalInput").ap()
    wfv_d = nc.dram_tensor("wfvT", [C, C], f32r, kind="ExternalInput").ap()
    wqk_d = nc.dram_tensor("wqkTg", [HW, J2], f32r, kind="ExternalInput").ap()
    wspa_d = nc.dram_tensor("wspaT", [HW, HW], f32r, kind="ExternalInput").ap()
    wfrq_d = nc.dram_tensor("wfrqT", [HW, HW], f32r, kind="ExternalInput").ap()
    qkb_d = qkbk_d = bspa_d = bfrq_d = bsv_d = bfv_d = None
    if has_qkb:
        qkb_d = nc.dram_tensor("qkb", [1, HW], f32r, kind="ExternalInput").ap()
        qkbk_d = nc.dram_tensor("qkbk", [1, HW], f32r,
                                kind="ExternalInput").ap()
    if has_bspa:
        bspa_d = nc.dram_tensor("bspa", [1, HW], f32r, kind="ExternalInput").ap()
    if has_bfrq:
        bfrq_d = nc.dram_tensor("bfrq", [1, HW], f32r, kind="ExternalInput").ap()
    if has_bsv:
        bsv_d = nc.dram_tensor("bsv", [1, C], f32r, kind="ExternalInput").ap()
    if has_bfv:
        bfv_d = nc.dram_tensor("bfv", [1, C], f32r, kind="ExternalInput").ap()
    os_d = nc.dram_tensor("os", [BPC, C, HW], f32, kind="ExternalOutput").ap()
    of_d = nc.dram_tensor("of", [BPC, C, HW], f32, kind="ExternalOutput").ap()

    Exp = mybir.ActivationFunctionType.Exp
    SUB = mybir.AluOpType.subtract
    MUL = mybir.AluOpType.mult
    ADD = mybir.AluOpType.add

    with tile.TileContext(nc) as tc:
        with tc.tile_pool(name="constp", bufs=1) as constp, \
             tc.tile_pool(name="wqkp", bufs=1) as wqkp, \
             tc.tile_pool(name="wsp", bufs=6) as wsp, \
             tc.tile_pool(name="xin", bufs=2) as xin, \
             tc.tile_pool(name="dat", bufs=2) as dat, \
             tc.tile_pool(name="attp", bufs=4) as attp, \
             tc.tile_pool(name="resp", bufs=2) as resp, \
             tc.tile_pool(name="small", bufs=4) as small, \
             tc.tile_pool(name="psS", bufs=2, space="PSUM") as psS, \
             tc.tile_pool(name="psL", bufs=2, space="PSUM") as psL, \
             tc.tile_pool(name="psBG", bufs=2, space="PSUM") as psBG:

            # ---- resident weights ----
            # Queues: ACT = small weights (fast HWDGE, land first) + sample-1
            # inputs + output stores; Pool(SWDGE) = wqk burst (its SEQ has no
            # early compute to clog); SP = sample-0 inputs + streamed ws.
            wsv_t = constp.tile([128, CC, C], f32r, name="wsv_t")
            nc.scalar.dma_start(out=wsv_t,
                                in_=wsv_d.rearrange("(kc p) c -> p kc c", p=128))
            wfv_t = constp.tile([128, CC, C], f32r, name="wfv_t")
            nc.scalar.dma_start(out=wfv_t,
                                in_=wfv_d.rearrange("(kc p) c -> p kc c", p=128))
            wcdc_t = constp.tile([128, 4, C], f32r, name="wcdc_t")
            nc.scalar.dma_start(out=wcdc_t,
                                in_=wcdc_d.rearrange("(kc p) c -> p kc c", p=128))
            ident_f = constp.tile([128, 128], f32, name="ident_f")
            make_identity(nc, ident_f)
            ones_t = None
            if any_mm_bias:
                ones_f = constp.tile([1, 128], f32, name="ones_f")
                nc.vector.memset(ones_f, 1.0)
                ones_t = constp.tile([1, 128], f32r, name="ones_t")
                nc.scalar.copy(out=ones_t, in_=ones_f)

            def _bias_tile(dram, n, nm):
                t = constp.tile([1, n], f32r, name=nm)
                nc.gpsimd.dma_start(out=t, in_=dram)
                return t

            qkb_t = _bias_tile(qkb_d, HW, "qkb_t") if has_qkb else None
            qkbk_t = _bias_tile(qkbk_d, HW, "qkbk_t") if has_qkb else None
            bspa_t = _bias_tile(bspa_d, HW, "bspa_t") if has_bspa else None
            bfrq_t = _bias_tile(bfrq_d, HW, "bfrq_t") if has_bfrq else None
            bsv_t = _bias_tile(bsv_d, C, "bsv_t") if has_bsv else None
            bfv_t = _bias_tile(bfv_d, C, "bfv_t") if has_bfv else None

            def _copy(eng, out, in_):
                if eng is nc.scalar:
                    eng.copy(out=out, in_=in_)
                else:
                    eng.tensor_copy(out=out, in_=in_)

            wqk_t = wqkp.tile([128, NCH, J2], f32r, name="wqk_t")
            for kc in range(NCH):
                nc.gpsimd.dma_start(out=wqk_t[:, kc, :],
                                    in_=wqk_d[ds(kc * 128, 128), :])

            def _samples_body():
                xsl, xfl, vtsl, vtfl, xsbl, xnTl, ql, kTl = \
                    [], [], [], [], [], [], [], []
                for b in range(BPC):
                    # sample 0 inputs on SP, sample 1 on ACT: two queues race
                    # ahead instead of one serial stream behind the weights
                    ieng = nc.sync if b == 0 else nc.scalar
                    xs_t = xin.tile([128, CC, HW], f32r, tag="xs", name=f"xs{b}")
                    ieng.dma_start(
                        out=xs_t,
                        in_=xs_d[b].rearrange("(cc p) n -> p cc n", p=128))
                    xf_t = xin.tile([128, CC, HW], f32r, tag="xf", name=f"xf{b}")
                    ieng.dma_start(
                        out=xf_t,
                        in_=xf_d[b].rearrange("(cc p) n -> p cc n", p=128))
                    xsl.append(xs_t)
                    xfl.append(xf_t)
                    vtsl.append(dat.tile([128, NCH, C], bf16, tag="vts",
                                         name=f"vts{b}"))
                    vtfl.append(dat.tile([128, NCH, C], bf16, tag="vtf",
                                         name=f"vtf{b}"))
                    # x_sb and (later) k_sb share the "xc" ring: x_sb{b} dies
                    # at C{b} before k_sb{b} is born at D{b}
                    xsbl.append(dat.tile([128, CC, HW], f32, tag="xc",
                                         name=f"xsb{b}"))
                    xnTl.append(dat.tile([128, NCH, C], f32r, tag="xnT",
                                         bufs=1, name=f"xnT{b}"))
                    ql.append(dat.tile([128, CC, HW], f32r, tag="q",
                                       bufs=1, name=f"q{b}"))
                    kTl.append(dat.tile([128, NCH, C], f32r, tag="kT",
                                        bufs=1, name=f"kT{b}"))
                # one kw ring slot: branch k+1's E overwrites after branch
                # k's F has consumed it (branches are sequential)

                def stageA(b):
                    # value projections vT_b = x_b.T @ w_bv.T  [hw, c]
                    # two mc chunks share one PSUM tile -> one grouped copy
                    for i, (src, wv, dst, bt) in enumerate(
                            ((xsl[b], wsv_t, vtsl[b], bsv_t),
                             (xfl[b], wfv_t, vtfl[b], bfv_t))):
                        for mg in range(NCH // 2):
                            ps = psS.tile([128, 512], f32, tag="ps", name="psa")
                            for half in range(2):
                                mc = mg * 2 + half
                                for kc in range(CC):
                                    nc.tensor.matmul(
                                        ps[:, ds(half * C, C)],
                                        src[:, kc, ts(mc, 128)], wv[:, kc, :],
                                        start=(kc == 0),
                                        stop=(kc == CC - 1 and bt is None))
                                if bt is not None:
                                    nc.tensor.matmul(ps[:, ds(half * C, C)],
                                                     ones_t, bt,
                                                     start=False, stop=True)
                            eng = nc.vector if mg % 2 == 0 else nc.scalar
                            _copy(eng, dst[:, ds(mg * 2, 2), :],
                                  ps.rearrange("p (a f) -> p a f", a=2))

                def stageB(b):
                    # x = w_cdc @ [xs; xf]  [c, hw]
                    for cc in range(CC):
                        for nn in range(2):
                            ps = psS.tile([128, 512], f32, tag="ps", name="psb")
                            for kc in range(4):
                                src = xsl[b] if kc < 2 else xfl[b]
                                nc.tensor.matmul(
                                    ps, wcdc_t[:, kc, ts(cc, 128)],
                                    src[:, kc % 2, ds(nn * 512, 512)],
                                    start=(kc == 0), stop=(kc == 3))
                            eng = nc.scalar if nn == 0 else nc.vector
                            _copy(eng, xsbl[b][:, cc, ds(nn * 512, 512)], ps)

                def stageLN(b):
                    # rstd = 1/sqrt(var+eps) on DVE only: Newton iteration
                    # y' = y*(1.5 - 0.5*v*y^2), seeded y0 = min(1/v, 1) so
                    # v*y0^2 <= 1 < 3 (convergent for any v > 0). Avoids the
                    # Act-engine Sqrt table load that thrashes against Exp.
                    x_sb = xsbl[b]
                    mvs = []
                    for cc in range(CC):
                        xr = x_sb[:, cc, :].rearrange("p (s f) -> p s f", s=2)
                        stats = small.tile([128, 2, 6], f32, tag="st",
                                           name="stats")
                        for s in range(2):
                            nc.vector.bn_stats(out=stats[:, s, :],
                                               in_=xr[:, s, :])
                        mv = small.tile([128, 2], f32, tag=f"mv{cc}",
                                        name="mv")
                        nc.vector.bn_aggr(out=mv, in_=stats)
                        mvs.append(mv)
                    veps = small.tile([128, 2], f32, tag="veps", name="veps")
                    for cc in range(CC):
                        nc.vector.tensor_scalar(
                            out=veps[:, cc:cc + 1], in0=mvs[cc][:, 1:2],
                            scalar1=EPS, scalar2=None, op0=ADD)
                    y = small.tile([128, 2], f32, tag="rstd", name="rstd")
                    nc.vector.reciprocal(out=y, in_=veps)
                    nc.vector.tensor_scalar_min(out=y, in0=y, scalar1=1.0)
                    t = small.tile([128, 2], f32, tag="nt", name="nt")
                    for _ in range(5):
                        nc.vector.tensor_tensor(out=t, in0=y, in1=y, op=MUL)
                        nc.vector.tensor_tensor(out=t, in0=t, in1=veps,
                                                op=MUL)
                        nc.vector.tensor_scalar(
                            out=t, in0=t, scalar1=-0.5, scalar2=1.5,
                            op0=MUL, op1=ADD)
                        nc.vector.tensor_tensor(out=y, in0=y, in1=t, op=MUL)
                    for cc in range(CC):
                        nc.vector.tensor_scalar(
                            out=x_sb[:, cc, :], in0=x_sb[:, cc, :],
                            scalar1=mvs[cc][:, 0:1], scalar2=y[:, cc:cc + 1],
                            op0=SUB, op1=MUL)

                def stageC(b):
                    # xnT = xn.T: 4 transposes per PSUM tile, 1 grouped copy
                    for cc in range(CC):
                        for dg in range(2):
                            pt = psS.tile([128, 512], f32, tag="ps", name="pt")
                            for i in range(4):
                                dc = dg * 4 + i
                                nc.tensor.transpose(
                                    pt[:, ds(i * 128, 128)],
                                    xsbl[b][:, cc, ds(dc * 128, 128)],
                                    ident_f)
                            eng = nc.scalar if dg == 0 else nc.vector
                            _copy(eng,
                                  xnTl[b][:, ds(dg * 4, 4), ts(cc, 128)],
                                  pt.rearrange("p (a f) -> p a f", a=4))

                def stageD(b, k_sb):
                    # q,k = xn @ wqkTg (f32r fused weight loads are free, so
                    # the jj-inner stationary reload pattern costs nothing)
                    for cc in range(CC):
                        for half in range(2):
                            for jj in range(2):
                                ps = psS.tile([128, 512], f32, tag="ps",
                                              name="psd")
                                for dc in range(NCH):
                                    nc.tensor.matmul(
                                        ps, xnTl[b][:, dc, ts(cc, 128)],
                                        wqk_t[:, dc,
                                              ds(half * HW + jj * 512, 512)],
                                        start=(dc == 0),
                                        stop=(dc == NCH - 1 and not has_qkb))
                                if has_qkb:
                                    bt = qkb_t if half == 0 else qkbk_t
                                    nc.tensor.matmul(
                                        ps, ones_t, bt[:, ds(jj * 512, 512)],
                                        start=False, stop=True)
                                eng = nc.scalar if jj == 0 else nc.vector
                                tgt = ql[b] if half == 0 else k_sb
                                _copy(eng, tgt[:, cc, ds(jj * 512, 512)], ps)

                def stageKT(b, k_sb):
                    # kT = k.T via PE transposes (f32r fused loads)
                    for cc in range(CC):
                        for dg in range(2):
                            pt = psS.tile([128, 512], f32, tag="ps", name="ptk")
                            for i in range(4):
                                mc = dg * 4 + i
                                nc.tensor.transpose(
                                    pt[:, ds(i * 128, 128)],
                                    k_sb[:, cc, ds(mc * 128, 128)], ident_f)
                            eng = nc.scalar if dg == 0 else nc.vector
                            _copy(eng,
                                  kTl[b][:, ds(dg * 4, 4), ts(cc, 128)],
                                  pt.rearrange("p (a f) -> p a f", a=4))

                def branch(b, br):
                    wsd = wspa_d if br == 0 else wfrq_d
                    lb_t = bspa_t if br == 0 else bfrq_t
                    vt = vtsl[b] if br == 0 else vtfl[b]
                    out_d = os_d if br == 0 else of_d
                    x_res = xsl[b] if br == 0 else xfl[b]
                    kw = dat.tile([128, CC, HW], f32r, tag="kw", bufs=1,
                                  name=f"kw{b}_{br}")

                    # E: kw = k @ (scale*w_b.T); ws streamed from HBM on SP,
                    # mc-outer so each chunk is consumed as it lands
                    ws_tiles = []
                    for mc in range(NCH):
                        wst = wsp.tile([128, HW], f32r, tag="ws",
                                       name=f"ws{b}_{br}_{mc}")
                        nc.sync.dma_start(out=wst,
                                          in_=wsd[ds(mc * 128, 128), :])
                        ws_tiles.append(wst)
                    pse = [psBG.tile([128, HW], f32, tag="bg",
                                     name=f"pse{b}_{br}_{cc}")
                           for cc in range(CC)]
                    for mc in range(NCH):
                        for cc in range(CC):
                            for jj in range(2):
                                nc.tensor.matmul(
                                    pse[cc][:, ds(jj * 512, 512)],
                                    kTl[b][:, mc, ts(cc, 128)],
                                    ws_tiles[mc][:, ds(jj * 512, 512)],
                                    start=(mc == 0), stop=(mc == NCH - 1))
                    for cc in range(CC):
                        eng = nc.vector if cc == 0 else nc.scalar
                        _copy(eng, kw[:, cc, :], pse[cc])

                    # F/G pipelined per nk: logit halves -> exp(+rowsum
                    # halves) -> vtn; G(nk-1) emitted after F(nk) so the
                    # softmax chain of nk hides under G(nk-1)'s matmuls
                    psg = [psBG.tile([128, HW], f32, tag="bg",
                                     name=f"psg{b}_{br}_{cc}")
                           for cc in range(CC)]
                    # seed the G accumulators with the residual so the
                    # output path is a plain PSUM->SBUF copy (no DVE adds
                    # on the critical tail)
                    for cc in range(CC):
                        eng = nc.vector if cc == 0 else nc.scalar
                        _copy(eng, psg[cc], x_res[:, cc, :].bitcast(f32))
                    ets, vtns = [], []

                    def emit_F(nk):
                        rs = small.tile([128, 2], f32, tag="rs", name="rsum")
                        pair = []
                        for hh in range(2):
                            pl = psL.tile([128, 512], f32, tag="pl", name="pl")
                            for cc in range(CC):
                                nc.tensor.matmul(
                                    pl, ql[b][:, cc, ts(nk, 128)],
                                    kw[:, cc, ds(hh * 512, 512)],
                                    start=(cc == 0),
                                    stop=(cc == CC - 1 and lb_t is None))
                            if lb_t is not None:
                                nc.tensor.matmul(pl, ones_t,
                                                 lb_t[:, ds(hh * 512, 512)],
                                                 start=False, stop=True)
                            et = attp.tile([128, 512], f32r, tag="att",
                                           name=f"et{b}_{br}_{nk}_{hh}")
                            nc.scalar.activation(out=et, in_=pl, func=Exp,
                                                 accum_out=rs[:, hh:hh + 1])
                            pair.append(et)
                        ets.append(pair)
                        rtot = small.tile([128, 1], f32, tag="rt", name="rtot")
                        nc.vector.tensor_tensor(out=rtot, in0=rs[:, 0:1],
                                                in1=rs[:, 1:2], op=ADD)
                        nc.vector.reciprocal(out=rtot, in_=rtot)
                        vtn = attp.tile([128, C], f32r, tag="vtn", bufs=2,
                                        name="vtn")
                        nc.vector.tensor_scalar_mul(out=vtn, in0=vt[:, nk, :],
                                                    scalar1=rtot)
                        vtns.append(vtn)

                    def emit_G(nk):
                        for cc in range(CC):
                            for hh in range(2):
                                nc.tensor.matmul(
                                    psg[cc][:, ds(hh * 512, 512)],
                                    vtns[nk][:, ts(cc, 128)], ets[nk][hh],
                                    start=False, stop=(nk == NCH - 1),
                                    skip_group_check=True)

                    for nk in range(NCH):
                        emit_F(nk)
                        if nk > 0:
                            emit_G(nk - 1)
                    emit_G(NCH - 1)

                    # out = psg (residual already seeded): copy + store
                    for cc in range(CC):
                        for hh in range(2):
                            res = resp.tile([128, 512], f32, tag="res",
                                            name=f"res{b}_{br}_{cc}_{hh}")
                            eng = nc.vector if hh == 0 else nc.scalar
                            _copy(eng, res, psg[cc][:, ds(hh * 512, 512)])
                            nc.scalar.dma_start(
                                out=out_d[b, ds(cc * 128, 128),
                                          ds(hh * 512, 512)],
                                in_=res)

                # ---- schedule ----
                stageA(0); stageB(0); stageLN(0)
                stageA(1); stageB(1); stageLN(1)
                stageC(0)
                k_sb0 = dat.tile([128, CC, HW], f32, tag="xc", name="ksb0")
                stageD(0, k_sb0); stageKT(0, k_sb0)
                branch(0, 0)
                branch(0, 1)
                stageC(1)
                k_sb1 = dat.tile([128, CC, HW], f32, tag="xc", name="ksb1")
                stageD(1, k_sb1); stageKT(1, k_sb1)
                branch(1, 0)
                branch(1, 1)

            if reps == 1:
                _samples_body()
            elif isinstance(reps, tuple):      # ("unroll", R)
                for _rep in range(reps[1]):
                    _samples_body()
            else:
                with tc.For_i(0, reps, 1):
                    _samples_body()

    nc.compile()
    return nc


def _prep_base(w_cdc, w_sv, w_fv, ln_w, ln_b, w_qk, w_spa, b_spa,
               w_frq, b_frq, b_sv, b_fv):
    """Host-side weight prep shared by kernel() and the bench harness."""
    scale = float(HW) ** -0.5
    qkb = np.asarray(ln_b, np.float32) @ np.asarray(w_qk, np.float32).T
    flags = (bool(np.any(qkb)), bool(np.any(b_spa)), bool(np.any(b_frq)),
             bool(np.any(b_sv)), bool(np.any(b_fv)))
    base = {
        "wcdcT": _round_f32r(np.asarray(w_cdc, np.float32).T),
        "wsvT": _round_f32r(np.asarray(w_sv, np.float32).T),
        "wfvT": _round_f32r(np.asarray(w_fv, np.float32).T),
        "wqkTg": _round_f32r(np.asarray(w_qk, np.float32).T
                             * np.asarray(ln_w, np.float32)[:, None]),
        "wspaT": _round_f32r(np.asarray(w_spa, np.float32).T * scale),
        "wfrqT": _round_f32r(np.asarray(w_frq, np.float32).T * scale),
    }
    if flags[0]:
        base["qkb"] = _round_f32r(qkb[None, :HW])
        base["qkbk"] = _round_f32r(qkb[None, HW:])
    if flags[1]:
        base["bspa"] = _round_f32r(np.asarray(b_spa, np.float32)[None, :])
    if flags[2]:
        base["bfrq"] = _round_f32r(np.asarray(b_frq, np.float32)[None, :])
    if flags[3]:
        base["bsv"] = _round_f32r(np.asarray(b_sv, np.float32)[None, :])
    if flags[4]:
        base["bfv"] = _round_f32r(np.asarray(b_fv, np.float32)[None, :])
    return base, flags


def kernel(x_spa, x_freq, w_cdc, b_cdc, w_sv, b_sv, w_fv, b_fv,
           ln_w, ln_b, w_qk, w_spa, b_spa, w_frq, b_frq):
    # b_cdc is a per-row constant added before LayerNorm over that row: no-op.
    base, flags = _prep_base(w_cdc, w_sv, w_fv, ln_w, ln_b, w_qk,
                             w_spa, b_spa, w_frq, b_frq, b_sv, b_fv)
    if flags not in _CACHE:
        _CACHE[flags] = _build(flags)
    nc = _CACHE[flags]

    xs = _round_f32r(np.asarray(x_spa, np.float32).reshape(B, C, HW))
    xf = _round_f32r(np.asarray(x_freq, np.float32).reshape(B, C, HW))
    in_maps = []
    for c in range(NCORES):
        m = dict(base)
        m["xs"] = xs[c * BPC:(c + 1) * BPC]
        m["xf"] = xf[c * BPC:(c + 1) * BPC]
        in_maps.append(m)

    res = bass_utils.run_bass_kernel_spmd(nc, in_maps,
                                          core_ids=list(range(NCORES)))
    out_spa = np.concatenate([res.results[c]["os"] for c in range(NCORES)],
                             axis=0)
    out_frq = np.concatenate([res.results[c]["of"] for c in range(NCORES)],
                             axis=0)
    return (out_spa.reshape(B, C, H, W).astype(np.float32),
            out_frq.reshape(B, C, H, W).astype(np.float32))
